# revision 2
# baseline (speedup 1.0000x reference)
"""Hashed-weight MLP (1024-4096-4096-32000, batch 2048) on 8 TRN2 NeuronCores.

Problem: h = relu(x @ W0); h = relu(h @ W1); out = h @ W2, where each
W_l[i, j] = hw_l[(a_l*i + b_l*j + c_l) % N_l] is a virtual (ROBE-Z hashed)
weight gathered from a small parameter vector.

Column-parallel tensor parallelism on all three layers; each core owns a
1/8 column shard of every layer. Activations stay transposed
[features, batch]. GEMMs are bf16 with fp32 PSUM accumulation.

The virtual-weight gather is performed host-side during input prep (the
hash index map is static, so each core's weight shard is a plain
re-indexing of the parameter vector); the device streams the shards as
dense bf16 tiles. W2 (32 MB/core) tiles are loaded just-in-time, paced
by the consuming tile pool, so DMA engines are never flooded ahead of
latency-critical loads (the main bottleneck of the previous version).

Engine assignment: tensor=matmuls; vector=ReLU+PSUM drains (keeps them
off DMA-issuing queues); sync ring=x/activation loads+stores; scalar
ring=weight loads+output stores; gpsimd=collectives (chunked AllGathers
hidden behind compute).
"""
import sys
if "/opt/trn_rl_repo" not in sys.path:
    sys.path.insert(0, "/opt/trn_rl_repo")

import numpy as np
import ml_dtypes

import concourse.bass as bass
import concourse.bacc as bacc
import concourse.tile as tile
import concourse.mybir as mybir
from concourse.bass_utils import run_bass_kernel_spmd

N_CORES = 8
P = 128
NB = 512                      # batch tile (PSUM bank = 512 fp32)
BATCH = 2048
BT = BATCH // NB              # 4

LENS = [1024, 4096, 4096, 32000]
HASH_A = [9973, 10007, 10039]
HASH_B = [31013, 31019, 31039]
HASH_C = [557, 563, 569]
SIZES = [1048576, 1048576, 4194304]

JW = [512, 512, 4000]         # true per-core output shard width
WMAT = [512, 512, 4096]       # materialized width (L2 padded to 32 j-tiles)

BF = mybir.dt.bfloat16
F32 = mybir.dt.float32

RG = [list(range(N_CORES))]


def build_nc():
    nc = bacc.Bacc("TRN2", target_bir_lowering=False, debug=False,
                   num_devices=N_CORES)

    xT_d = nc.dram_tensor("xT", [LENS[0], BATCH], BF, kind="ExternalInput").ap()
    w0_d = nc.dram_tensor("w0m", [1024, 512], BF, kind="ExternalInput").ap()
    w1_d = nc.dram_tensor("w1m", [4096, 512], BF, kind="ExternalInput").ap()
    w2_d = nc.dram_tensor("w2m", [4096, 4096], BF, kind="ExternalInput").ap()
    h1c = [nc.dram_tensor(f"h1c{b}", [512, NB], BF).ap() for b in range(BT)]
    h1f = [nc.dram_tensor(f"h1f{b}", [4096, NB], BF, addr_space="Shared").ap()
           for b in range(BT)]
    h2c = [nc.dram_tensor(f"h2c{p}", [512, 2 * NB], BF).ap() for p in range(2)]
    h2f = [nc.dram_tensor(f"h2f{p}", [4096, 2 * NB], BF, addr_space="Shared").ap()
           for p in range(2)]
    out_d = nc.dram_tensor("outT", [4096, BATCH], BF, kind="ExternalOutput").ap()

    with tile.TileContext(nc) as tc, \
         tc.tile_pool(name="ps", bufs=8, space="PSUM") as psp, \
         tc.tile_pool(name="w2p", bufs=32) as w2p:
        with tc.tile_pool(name="l01", bufs=1) as l01p, \
             tc.tile_pool(name="act", bufs=10) as actp:
            # W0 on scalar ring, x on sync ring: L0 starts ASAP
            w0sb = [l01p.tile([P, 512], BF, name=f"w0sb{kt}") for kt in range(8)]
            for kt in range(8):
                nc.scalar.dma_start(out=w0sb[kt][:],
                                    in_=w0_d[kt * P:(kt + 1) * P, :])
            xsb = [l01p.tile([P, BATCH], BF, name=f"xsb{kt}") for kt in range(8)]
            for kt in range(8):
                nc.sync.dma_start(out=xsb[kt][:], in_=xT_d[kt * P:(kt + 1) * P, :])
            w1sb = [l01p.tile([P, 512], BF, name=f"w1sb{kt}") for kt in range(32)]
            for kt in range(32):
                nc.scalar.dma_start(out=w1sb[kt][:],
                                    in_=w1_d[kt * P:(kt + 1) * P, :])

            # ---- L0 ----
            for b in range(BT):
                pss = [psp.tile([P, NB], F32, tag="ps", name=f"ps0_{b}_{j}")
                       for j in range(4)]
                for kt in range(8):
                    for j in range(4):
                        nc.tensor.matmul(
                            out=pss[j][:],
                            lhsT=w0sb[kt][:, j * P:(j + 1) * P],
                            rhs=xsb[kt][:, b * NB:(b + 1) * NB],
                            start=(kt == 0), stop=(kt == 7))
                for j in range(4):
                    hsb = actp.tile([P, NB], BF, tag="act", name=f"h1s_{b}_{j}")
                    nc.vector.tensor_scalar_max(out=hsb[:], in0=pss[j][:],
                                                scalar1=0.0)
                    nc.sync.dma_start(out=h1c[b][j * P:(j + 1) * P, :],
                                      in_=hsb[:])
                nc.gpsimd.collective_compute(
                    "AllGather", mybir.AluOpType.bypass, replica_groups=RG,
                    ins=[h1c[b].opt()], outs=[h1f[b].opt()])

            # W2 jg0 slab loads stream in behind L0/L1 on the scalar ring
            slab = [[None] * 32 for _ in range(4)]
            for t in range(32):
                slab[0][t] = w2p.tile([P, 1024], BF, tag="w2t", name=f"w2_0_{t}")
                nc.scalar.dma_start(out=slab[0][t][:],
                                    in_=w2_d[t * P:(t + 1) * P, 0:1024])

            # ---- L1 ----
            with tc.tile_pool(name="l1r", bufs=8) as l1rp:
                for b in range(BT):
                    pss = [psp.tile([P, NB], F32, tag="ps", name=f"ps1_{b}_{j}")
                           for j in range(4)]
                    for kt in range(32):
                        rhs = l1rp.tile([P, NB], BF, tag="l1rhs",
                                        name=f"l1r_{b}_{kt}")
                        nc.sync.dma_start(out=rhs[:],
                                          in_=h1f[b][kt * P:(kt + 1) * P, :])
                        for j in range(4):
                            nc.tensor.matmul(
                                out=pss[j][:],
                                lhsT=w1sb[kt][:, j * P:(j + 1) * P],
                                rhs=rhs[:],
                                start=(kt == 0), stop=(kt == 31))
                    for j in range(4):
                        hsb = actp.tile([P, NB], BF, tag="act",
                                        name=f"h2s_{b}_{j}")
                        nc.vector.tensor_scalar_max(out=hsb[:], in0=pss[j][:],
                                                    scalar1=0.0)
                        nc.sync.dma_start(
                            out=h2c[b // 2][j * P:(j + 1) * P,
                                            (b % 2) * NB:(b % 2 + 1) * NB],
                            in_=hsb[:])
                    if b % 2 == 1:
                        nc.gpsimd.collective_compute(
                            "AllGather", mybir.AluOpType.bypass,
                            replica_groups=RG,
                            ins=[h2c[b // 2].opt()], outs=[h2f[b // 2].opt()])

        # ---- L2: h2 fully SBUF-resident (read once; DMA power/traffic
        # minimal under the GPIO power throttle), W2 tiles streamed in
        # place, lhsT reused x2 (4j x 2b = 8 PSUM banks per pass).
        with tc.tile_pool(name="l2f", bufs=64) as l2fp, \
             tc.tile_pool(name="ost", bufs=4) as ostp:
            h2sb = [[None] * 32 for _ in range(2)]
            for bp in range(2):
                for kt in range(32):
                    t = l2fp.tile([P, 2 * NB], BF, tag="l2full",
                                  name=f"h2sb_{bp}_{kt}")
                    nc.sync.dma_start(out=t[:],
                                      in_=h2f[bp][kt * P:(kt + 1) * P, :])
                    h2sb[bp][kt] = t
            for jg in range(4):
                for bp in range(2):
                    for jh in range(2):
                        pss = [psp.tile([P, NB], F32, tag="ps",
                                        name=f"ps2_{jg}_{bp}_{jh}_{g}")
                               for g in range(8)]
                        for kt in range(32):
                            for j in range(4):
                                for b in range(2):
                                    nc.tensor.matmul(
                                        out=pss[j * 2 + b][:],
                                        lhsT=slab[jg][kt][
                                            :, (jh * 4 + j) * P:
                                               (jh * 4 + j + 1) * P],
                                        rhs=h2sb[bp][kt][:, b * NB:(b + 1) * NB],
                                        start=(kt == 0), stop=(kt == 31))
                            if bp == 1 and jh == 1 and jg < 3:
                                # next j-group's tile reuses this slot as soon
                                # as its last consumer retires (pool-paced)
                                nt = w2p.tile([P, 1024], BF, tag="w2t",
                                              name=f"w2_{jg + 1}_{kt}")
                                nc.scalar.dma_start(
                                    out=nt[:],
                                    in_=w2_d[kt * P:(kt + 1) * P,
                                             (jg + 1) * 1024:(jg + 2) * 1024])
                                slab[jg + 1][kt] = nt
                        for j in range(4):
                            osb = ostp.tile([P, 2 * NB], BF, tag="l2out",
                                            name=f"l2o_{jg}_{bp}_{jh}_{j}")
                            for b in range(2):
                                nc.vector.tensor_copy(
                                    out=osb[:, b * NB:(b + 1) * NB],
                                    in_=pss[j * 2 + b][:])
                            jr = (jg * 8 + jh * 4 + j) * P
                            nc.gpsimd.dma_start(
                                out=out_d[jr:jr + P,
                                          2 * bp * NB:(2 * bp + 2) * NB],
                                in_=osb[:])

    nc.compile()
    return nc


_NC_CACHE = None


def _get_nc():
    global _NC_CACHE
    if _NC_CACHE is None:
        _NC_CACHE = build_nc()
    return _NC_CACHE


def _prep_inputs(x, hw0, hw1, hw2):
    """Host prep: transpose x; gather each core's dense weight shards from
    the hashed parameter vectors (static index map)."""
    x = np.asarray(x, np.float32)
    hws = [np.asarray(hw0, np.float32), np.asarray(hw1, np.float32),
           np.asarray(hw2, np.float32)]
    xT = np.ascontiguousarray(x.T).astype(ml_dtypes.bfloat16)

    # W[i, j] = hw[(a*i + b*j + ch) % N] = hw_bb[(u0 + q*i) % N + j] where
    # hw_bb[t] = hw[(b*t) % N] (periodic, b odd => invertible mod N=2^k).
    # Materialize each core's shard as sliding windows of a periodic slice.
    w_shards = [[None] * 3 for _ in range(N_CORES)]
    for l in range(3):
        N = SIZES[l]
        a, b, ch = HASH_A[l], HASH_B[l], HASH_C[l]
        binv = pow(b, -1, N)
        q = (binv * a) % N
        u0 = (binv * ch) % N
        t = np.arange(N + WMAT[l] + (N_CORES - 1) * JW[l], dtype=np.int64)
        hb = hws[l][(b * (u0 + t)) % N].astype(ml_dtypes.bfloat16)
        starts = (q * np.arange(LENS[l], dtype=np.int64)) % N
        cols = np.arange(WMAT[l], dtype=np.int64)
        for c in range(N_CORES):
            w_shards[c][l] = hb[(starts[:, None] + c * JW[l]) + cols[None, :]]
    in_maps = []
    for c in range(N_CORES):
        in_maps.append({
            "xT": xT,
            "w0m": w_shards[c][0],
            "w1m": w_shards[c][1],
            "w2m": w_shards[c][2],
        })
    return in_maps


def kernel(x, hw0, hw1, hw2, trace=False):
    nc = _get_nc()
    in_maps = _prep_inputs(x, hw0, hw1, hw2)
    res = run_bass_kernel_spmd(nc, in_maps, list(range(N_CORES)), trace=trace)
    outs = [res.results[c]["outT"][:JW[2], :].astype(np.float32)
            for c in range(N_CORES)]
    full = np.concatenate(outs, axis=0)        # [32000, 2048]
    out = np.ascontiguousarray(full.T)         # [2048, 32000] fp32
    kernel.last_results = res
    return out
